# revision 2
# baseline (speedup 1.0000x reference)
"""Trainium2 Bass kernel for BiFormer-style sparse window attention routing
(nn_BA_28784870818378), SPMD across 8 NeuronCores, two-launch design.

Host contract: kernel(x, w_qkv, b_qkv) takes the FULL inputs
(x (2,192,256,256) f32, w_qkv (192,576) f32, b_qkv (576,) f32) and returns
the FULL output (2, 1024, 4, 64, 192) f32.

Launch 1 (all 8 cores, disjoint slices): core c owns image-row band
[32c, 32c+32) of BOTH batches (1/8 of x, 12.5MB). It computes
  - the per-pixel v projection in fp16 (fp16 matmul inputs), and
  - the fp32 per-window channel sums (for the routing means),
for its 128 windows per batch. ~19MB DMA per core.

Host relay: pure layout work (np transpose/concat) assembles the full
v16[b] (1024, 12288) gather table and xbar[b] (192, 1024) window sums.

Launch 2 (core c: batch c//4, query-quarter c%4): computes q/k window
projections and logits in exact fp32, top-4 window ids per query row
(vector max/max_index), then indirect-DMA gathers the selected v16 rows
and writes its quarter of the output (fp16). ~51MB DMA per core.

Host finally concatenates the 8 output shards and casts fp16 -> fp32.
"""

import numpy as np

import concourse.bass as bass
import concourse.mybir as mybir
from concourse.bass import IndirectOffsetOnAxis
from concourse.tile import TileContext
from concourse.vector_clock import ScopedClock


_orig_commit_and_lower = TileContext._commit_and_lower


def _split_commit_and_lower(self, inst, original_block, old_bb_map, bb_to_exit_bb):
    si = inst.sync_info
    if si is not None and si.on_wait is not None and len(si.on_wait) > 1:
        waits = list(si.on_wait)
        updates = list(si.on_update) if si.on_update else []
        inst.sync_info = mybir.SyncInfo(on_wait=[waits[-1]], on_update=updates)
        for w in waits[:-1]:
            nop = mybir.InstNoOp(
                name=self.nc.get_next_instruction_name(),
                engine=inst.engine,
                ins=[],
                outs=[],
                sync_info=mybir.SyncInfo(on_wait=[w], on_update=[]),
                bass_nofuse=True,
            )
            _orig_commit_and_lower(self, nop, original_block, old_bb_map, bb_to_exit_bb)
    return _orig_commit_and_lower(self, inst, original_block, old_bb_map, bb_to_exit_bb)


def _patched_drain_and_barrier(self, tick_clock, wait_clock):
    nop0 = self.nc.sync.nop(nofuse=True, hint="drain_waits")
    wait_clock.add_sem_waits(nop0.ins, ScopedClock({None: tick_clock.global_clock}))
    si = nop0.ins.sync_info
    waits = list(si.on_wait) if si is not None and si.on_wait else []
    if len(waits) > 1:
        nop0.ins.sync_info = mybir.SyncInfo(on_wait=[waits[0]], on_update=[])
        for w in waits[1:]:
            nopi = self.nc.sync.nop(nofuse=True, hint="drain_waits")
            nopi.ins.sync_info = mybir.SyncInfo(on_wait=[w], on_update=[])
    self.nc.sync.drain()

    self.nc.all_engine_barrier()
    assert self.sems is not None
    popped = self.nc._tile_sem_poison_stack.pop()
    assert popped is self._sem_poison
    self.nc.clear_and_free_semaphores(list(self.sems.allocated().values()))
    self.nc.all_engine_barrier()


def _apply_walrus_workarounds():
    TileContext._commit_and_lower = _split_commit_and_lower
    TileContext._drain_and_barrier = _patched_drain_and_barrier


F32 = mybir.dt.float32
F32R = mybir.dt.float32r
F16 = mybir.dt.float16
U32 = mybir.dt.uint32

C = 192            # channels
H = W = 256
WIN = 8
NH = NW = H // WIN  # 32
NWIN = NH * NW      # 1024 windows per batch
SHW = WIN * WIN     # 64 pixels per window
TOPK = 4
D = 192            # v dim
QK = 192
BLK = SHW * D      # 12288 elements per gathered block
SCALE = QK ** -0.5

SLICE = H * W // 8  # 8192 pixels per core per batch
SLAB = 2048         # pixels per processing slab
NSLAB = SLICE // SLAB  # 4 slabs per batch


_apply_walrus_workarounds()


def build_l1():
    """Launch 1: x slice -> v16 slabs (fp16, [out_ch, pix]) + window sums.

    x is loaded as true fp32 (the window sums feed the routing and must be
    exact fp32 — an f32r-typed load would round in flight and flip near-tie
    top-k selections). The v matmul takes fp16 casts of x, spread across
    ACT (xa) and DVE (xb); window sums run on DVE (xa) and Pool (xb).
    """
    nc = bass.Bass("TRN2")
    x = nc.dram_tensor("x", [2 * C, SLICE], F32, kind="ExternalInput")
    wv = nc.dram_tensor("wv", [C, D], F32, kind="ExternalInput")
    bv = nc.dram_tensor("bv", [D, 1], F32, kind="ExternalInput")
    wqk = nc.dram_tensor("wqk", [C, 2 * QK], F32, kind="ExternalInput")
    bqk = nc.dram_tensor("bqk", [2 * QK, 1], F32, kind="ExternalInput")
    vq = nc.dram_tensor("vq", [2 * D, SLICE], F16, kind="ExternalOutput")
    qw = nc.dram_tensor("qw", [2 * QK, 128], F32, kind="ExternalOutput")
    kw = nc.dram_tensor("kw", [2 * QK, 128], F32, kind="ExternalOutput")

    NBUF = 6

    with TileContext(nc) as tc:
        with (
            tc.tile_pool(name="const", bufs=1) as cp,
            tc.tile_pool(name="slab", bufs=1) as sp,
            tc.tile_pool(name="psa", bufs=3, space="PSUM") as ppa,
            tc.tile_pool(name="psb", bufs=3, space="PSUM") as ppb,
            tc.tile_pool(name="psp", bufs=2, space="PSUM") as ppp,
        ):
            # v-projection weights (fp16); wvb row 64 holds the bias (ones trick)
            wva = cp.tile([128, D], F32, tag="wva")
            wvb = cp.tile([65, D], F32, tag="wvb")
            nc.sync.dma_start(out=wva[:], in_=wv[0:128, :])
            nc.sync.dma_start(out=wvb[0:64, :], in_=wv[128:192, :])
            nc.sync.dma_start(out=wvb[64:65, :], in_=bv[:, 0:1].rearrange("d one -> (one) d"))
            wva16 = cp.tile([128, D], F16, tag="wva16")
            wvb16 = cp.tile([65, D], F16, tag="wvb16")
            nc.vector.tensor_copy(out=wva16[:], in_=wva[:])
            nc.vector.tensor_copy(out=wvb16[:], in_=wvb[:])

            # q/k projection weights (fp32, exact) + biases
            wqa = cp.tile([128, 2 * QK], F32, tag="wqa")
            wqb = cp.tile([64, 2 * QK], F32, tag="wqb")
            nc.sync.dma_start(out=wqa[:], in_=wqk[0:128, :])
            nc.sync.dma_start(out=wqb[:], in_=wqk[128:192, :])
            bqa = cp.tile([128, 1], F32, tag="bqa")
            bqb = cp.tile([64, 1], F32, tag="bqb")
            bka = cp.tile([128, 1], F32, tag="bka")
            bkb = cp.tile([64, 1], F32, tag="bkb")
            nc.sync.dma_start(out=bqa[:], in_=bqk[0:128, :])
            nc.sync.dma_start(out=bqb[:], in_=bqk[128:192, :])
            nc.sync.dma_start(out=bka[:], in_=bqk[192:320, :])
            nc.sync.dma_start(out=bkb[:], in_=bqk[320:384, :])
            # q bias pre-scaled by SCALE (logits use scale*q_win)
            nc.scalar.mul(bqa[:], bqa[:], SCALE)
            nc.scalar.mul(bqb[:], bqb[:], SCALE)

            # per-window channel sums: cols = 128*b + 32*s + nw
            xbar_a = cp.tile([128, 256], F32, tag="xbar_a")
            xbar_b = cp.tile([64, 256], F32, tag="xbar_b")

            # q/k projection accumulators (same column layout as xbar)
            qwt = (
                cp.tile([128, 256], F32, tag="qwt_a", name="qwt_a"),
                cp.tile([64, 256], F32, tag="qwt_b", name="qwt_b"),
            )
            kwt = (
                cp.tile([128, 256], F32, tag="kwt_a", name="kwt_a"),
                cp.tile([64, 256], F32, tag="kwt_b", name="kwt_b"),
            )

            xa_t = [sp.tile([128, SLAB], F32, tag=f"xa{i}", name=f"xa{i}") for i in range(NBUF)]
            xb_t = [sp.tile([65, SLAB], F32, tag=f"xb{i}", name=f"xb{i}") for i in range(NBUF)]
            xa16_t = [sp.tile([128, SLAB], F16, tag=f"xa16_{i}", name=f"xa16_{i}") for i in range(NBUF)]
            xb16_t = [sp.tile([65, SLAB], F16, tag=f"xb16_{i}", name=f"xb16_{i}") for i in range(NBUF)]
            sta_t = [sp.tile([128, SLAB], F16, tag=f"sta{i}", name=f"sta{i}") for i in range(NBUF)]
            stb_t = [sp.tile([64, SLAB], F16, tag=f"stb{i}", name=f"stb{i}") for i in range(NBUF)]
            for i in range(NBUF):
                nc.gpsimd.memset(xb_t[i][64:65, :], 1.0)

            for b in range(2):
                for s in range(NSLAB):
                    t = 4 * b + s
                    xa, xb = xa_t[t % NBUF], xb_t[t % NBUF]
                    xa16, xb16 = xa16_t[t % NBUF], xb16_t[t % NBUF]
                    sta, stb = sta_t[t % NBUF], stb_t[t % NBUF]
                    csl = slice(SLAB * s, SLAB * (s + 1))
                    nc.sync.dma_start(out=xa[:], in_=x[192 * b : 192 * b + 128, csl])
                    nc.sync.dma_start(out=xb[0:64, :], in_=x[192 * b + 128 : 192 * b + 192, csl])

                    # window sums for routing (exact fp32)
                    xar = xa[:].rearrange("p (dh nw dw) -> p nw dh dw", dh=8, nw=32, dw=8)
                    xbr = xb[0:64].rearrange("p (dh nw dw) -> p nw dh dw", dh=8, nw=32, dw=8)
                    osl = slice(128 * b + 32 * s, 128 * b + 32 * (s + 1))
                    nc.vector.reduce_sum(out=xbar_a[:, osl], in_=xar, axis=mybir.AxisListType.XY)
                    nc.vector.reduce_sum(out=xbar_b[:, osl], in_=xbr, axis=mybir.AxisListType.XY)

                    # fp16 casts for the matmul inputs (split DVE/ACT/Pool)
                    nc.vector.tensor_copy(out=xa16[:, 0:1024], in_=xa[:, 0:1024])
                    nc.scalar.copy(out=xa16[:, 1024:2048], in_=xa[:, 1024:2048])
                    nc.gpsimd.tensor_copy(out=xb16[:], in_=xb[:])

                    # v = x @ Wv + bv (bias via xb ones row), fp16 matmul
                    for j, n0 in enumerate(range(0, SLAB, 512)):
                        pa = ppa.tile([128, 512], F32, tag="pa", name="pa")
                        pb = ppb.tile([64, 512], F32, tag="pb", name="pb")
                        xach = xa16[:, n0 : n0 + 512]
                        xbch = xb16[:, n0 : n0 + 512]
                        nc.tensor.matmul(pa[:], lhsT=wva16[:, 0:128], rhs=xach,
                                         start=True, stop=False)
                        nc.tensor.matmul(pa[:], lhsT=wvb16[:, 0:128], rhs=xbch,
                                         start=False, stop=True)
                        nc.tensor.matmul(pb[:], lhsT=wva16[:, 128:192], rhs=xach,
                                         start=True, stop=False)
                        nc.tensor.matmul(pb[:], lhsT=wvb16[:, 128:192], rhs=xbch,
                                         start=False, stop=True)
                        # evac spread (Pool can't read PSUM): 1 DVE + 7 ACT per slab
                        if j == 0:
                            nc.vector.tensor_copy(out=sta[:, n0 : n0 + 512], in_=pa[:])
                            nc.scalar.copy(out=stb[:, n0 : n0 + 512], in_=pb[:])
                        else:
                            nc.scalar.copy(out=sta[:, n0 : n0 + 512], in_=pa[:])
                            nc.scalar.copy(out=stb[:, n0 : n0 + 512], in_=pb[:])

                    nc.gpsimd.dma_start(out=vq[192 * b : 192 * b + 128, csl], in_=sta[:])
                    nc.gpsimd.dma_start(out=vq[192 * b + 128 : 192 * b + 192, csl], in_=stb[:])

                    # q/k window projections for this slab's 32 windows (exact
                    # fp32, scale and bias folded in) — incremental, so only
                    # the last slab's tiny projection sits on the critical tail
                    COPY = mybir.ActivationFunctionType.Identity
                    for col0, pts, sc, (b_a, b_b) in (
                        (0, qwt, SCALE / SHW, (bqa, bqb)),
                        (QK, kwt, 1.0 / SHW, (bka, bkb)),
                    ):
                        for (pt, d0, dn, bias) in ((pts[0], 0, 128, b_a), (pts[1], 128, 64, b_b)):
                            ps = ppp.tile([dn, 32], F32, tag="psproj", name="psproj")
                            nc.tensor.matmul(
                                ps[:], lhsT=wqa[:, col0 + d0 : col0 + d0 + dn],
                                rhs=xbar_a[:, osl], start=True, stop=False,
                            )
                            nc.tensor.matmul(
                                ps[:], lhsT=wqb[:, col0 + d0 : col0 + d0 + dn],
                                rhs=xbar_b[:, osl], start=False, stop=True,
                            )
                            nc.scalar.activation(
                                out=pt[:, 128 * b + 32 * s : 128 * b + 32 * (s + 1)],
                                in_=ps[:], func=COPY, bias=bias[:, 0:1], scale=sc,
                            )

            for pts, t_dram in ((qwt, qw), (kwt, kw)):
                for (pt, d0, dn) in ((pts[0], 0, 128), (pts[1], 128, 64)):
                    for b in range(2):
                        nc.sync.dma_start(
                            out=t_dram[192 * b + d0 : 192 * b + d0 + dn, :],
                            in_=pt[:, 128 * b : 128 * (b + 1)],
                        )

    return nc


def build_l2():
    """Launch 2: logits (fp32) + top-4 + indirect gather of v16 rows.

    q/k window projections arrive precomputed from launch 1 (scale and bias
    already folded in), so the serial head before the first gather is just
    two small loads, one row-tile of logits, and the top-4 index plumbing.
    """
    nc = bass.Bass("TRN2")
    vpix = nc.dram_tensor("vpix", [NWIN, BLK], F16, kind="ExternalInput")
    qwin = nc.dram_tensor("qwin", [C, 256], F32, kind="ExternalInput")
    kwin = nc.dram_tensor("kwin", [C, NWIN], F32, kind="ExternalInput")
    out = nc.dram_tensor("out", [NWIN, BLK], F16, kind="ExternalOutput")
    idxf = nc.dram_tensor("idxf", [2 * 512, 1], U32, kind="ExternalOutput")

    with TileContext(nc) as tc:
        with (
            tc.tile_pool(name="const", bufs=1) as cp,
            tc.tile_pool(name="psl", bufs=2, space="PSUM") as ppl,
            tc.tile_pool(name="small", bufs=2) as p3,
            tc.tile_pool(name="gat", bufs=4) as gp,
        ):
            qta = cp.tile([128, 256], F32, tag="qta")
            qtb = cp.tile([64, 256], F32, tag="qtb")
            kta = cp.tile([128, NWIN], F32, tag="kta")
            ktb = cp.tile([64, NWIN], F32, tag="ktb")
            nc.scalar.dma_start(out=qta[:], in_=qwin[0:128, :])
            nc.scalar.dma_start(out=qtb[:], in_=qwin[128:192, :])
            nc.scalar.dma_start(out=kta[:], in_=kwin[0:128, :])
            nc.scalar.dma_start(out=ktb[:], in_=kwin[128:192, :])

            # PE p-state warmup: keep the tensor engine busy >3us during the
            # loads so the first logits matmuls run at full clock
            wu = cp.tile([1, 512], mybir.dt.bfloat16, tag="wu")
            nc.gpsimd.memset(wu[:], 0.0)
            with tc.tile_pool(name="pswu", bufs=1, space="PSUM") as ppw:
                pw = ppw.tile([1, 512], F32, tag="pw", name="pw")
                for _ in range(8):
                    nc.tensor.matmul(pw[:], lhsT=wu[:, 0:1], rhs=wu[:],
                                     start=True, stop=True)

            # ---- per row-tile: logits, top-4, gather --------------------------
            idv = idxf[:].rearrange("(t p k) one -> t p (k one)", p=128, k=TOPK)

            for t in range(2):
                qsl = slice(128 * t, 128 * (t + 1))
                # logits into one 2-bank PSUM tile; max reads PSUM directly
                lg = ppl.tile([128, NWIN], F32, tag="plog", name="ps_log")
                for m0 in range(0, NWIN, 512):
                    nc.tensor.matmul(
                        lg[:, m0 : m0 + 512], lhsT=qta[:, qsl],
                        rhs=kta[:, m0 : m0 + 512], start=True, stop=False,
                    )
                    nc.tensor.matmul(
                        lg[:, m0 : m0 + 512], lhsT=qtb[:, qsl],
                        rhs=ktb[:, m0 : m0 + 512], start=False, stop=True,
                    )

                mx8 = p3.tile([128, 8], F32, tag="mx8", name="mx8")
                mi8 = p3.tile([128, 8], U32, tag="mi8", name="mi8")
                nc.vector.max(out=mx8[:], in_=lg[:])
                nc.vector.max_index(out=mi8[:], in_max=mx8[:], in_values=lg[:])
                nc.sync.dma_start(out=idv[t], in_=mi8[:, 0:TOPK])

                # gather groups keyed by (half, slot): offsets come straight
                # from mi8 partitions — no DRAM round-trip on the critical
                # path. Offset APs must start at partition 0 (base-64 offset
                # APs hard-fault the DMA engines), so the upper half bounces
                # through a base-0 tile first.
                mi8b = p3.tile([64, 8], U32, tag="mi8b", name="mi8b")
                nc.sync.dma_start(out=mi8b[:], in_=mi8[64:128, :])
                ov = out[512 * t : 512 * (t + 1), :].rearrange(
                    "(h q k) d -> h k q d", h=2, q=64, k=TOPK
                )
                for h in range(2):
                    src_off = mi8 if h == 0 else mi8b
                    for k in range(TOPK):
                        gt = gp.tile([64, BLK], F16, tag="gt", name="gt")
                        nc.gpsimd.indirect_dma_start(
                            out=gt[:], out_offset=None, in_=vpix[:],
                            in_offset=IndirectOffsetOnAxis(
                                ap=src_off[0:64, k : k + 1], axis=0
                            ),
                        )
                        if k % 2 == 0:
                            nc.scalar.dma_start(out=ov[h, k], in_=gt[:])
                        else:
                            nc.sync.dma_start(out=ov[h, k], in_=gt[:])

    return nc


# ---------------------------------------------------------------------------
# Host glue
# ---------------------------------------------------------------------------

_NC_CACHE = None


def _get_ncs():
    global _NC_CACHE
    if _NC_CACHE is None:
        _NC_CACHE = (build_l1(), build_l2())
    return _NC_CACHE


def kernel(x, w_qkv, b_qkv):
    from concourse.bass_utils import run_bass_kernel_spmd

    x = np.ascontiguousarray(np.asarray(x, dtype=np.float32))
    w_qkv = np.ascontiguousarray(np.asarray(w_qkv, dtype=np.float32))
    b_qkv = np.ascontiguousarray(np.asarray(b_qkv, dtype=np.float32))

    nc1, nc2 = _get_ncs()

    # ---- launch 1: v16 + q/k window projections on 1/8 row-bands -----------
    wv = np.ascontiguousarray(w_qkv[:, 384:576])
    bv = np.ascontiguousarray(b_qkv[384:576].reshape(D, 1))
    wqk = np.ascontiguousarray(w_qkv[:, 0:384])
    bqk = np.ascontiguousarray(b_qkv[0:384].reshape(2 * QK, 1))
    in1 = []
    for c in range(8):
        xs = np.ascontiguousarray(
            x[:, :, 32 * c : 32 * (c + 1), :].reshape(2 * C, SLICE)
        )
        in1.append({"x": xs, "wv": wv, "bv": bv, "wqk": wqk, "bqk": bqk})
    res1 = run_bass_kernel_spmd(nc1, in1, core_ids=list(range(8)))

    # ---- host relay: pure layout work --------------------------------------
    v16 = np.empty((2, NWIN, BLK), dtype=np.float16)
    qwf = np.empty((2, C, NWIN), dtype=np.float32)
    kwf = np.empty((2, C, NWIN), dtype=np.float32)
    for c in range(8):
        vq = res1.results[c]["vq"].reshape(2, D, NSLAB, 8, 32, 8)
        # [b, ch, s, dh, nw, dw] -> [b, (s nw), (dh dw), ch]
        vb = np.ascontiguousarray(vq.transpose(0, 2, 4, 3, 5, 1))
        v16[:, 128 * c : 128 * (c + 1)] = vb.reshape(2, 128, BLK)
        qwf[:, :, 128 * c : 128 * (c + 1)] = res1.results[c]["qw"].reshape(2, C, 128)
        kwf[:, :, 128 * c : 128 * (c + 1)] = res1.results[c]["kw"].reshape(2, C, 128)

    # ---- launch 2: routing + gather ----------------------------------------
    in2 = []
    for core in range(8):
        b = core // 4
        q = core % 4
        in2.append(
            {
                "vpix": v16[b],
                "qwin": np.ascontiguousarray(qwf[b][:, 256 * q : 256 * (q + 1)]),
                "kwin": np.ascontiguousarray(kwf[b]),
            }
        )
    res2 = run_bass_kernel_spmd(nc2, in2, core_ids=list(range(8)))

    full = np.empty((2, NWIN, TOPK, SHW, D), dtype=np.float32)
    for core in range(8):
        b = core // 4
        q = core % 4
        r = res2.results[core]["out"].astype(np.float32).reshape(256, TOPK, SHW, D)
        full[b, 256 * q : 256 * (q + 1)] = r
    return full
